# revision 8
# baseline (speedup 1.0000x reference)
"""Trainium2 Bass kernel for AbsDiagNet.

Reference computation (T=256, B=128, I=512, H=2048, O=512):
    proj = einsum('tbi,hi->tbh', X, W_IH)
    h_0 = 0;  h_t = |proj_t + HH * h_{t-1}|   (elementwise over [B, H])
    Y = h_T @ W_HO.T + b_HO                   -> [B, O]

Strategy: data-parallel over batch across 8 cores (B_local = 16), params
replicated.  All operand transposes are done host-side during sharding so the
device only runs matmuls (float32r, full PE rate), the serial DVE recurrence,
and ACT-engine PSUM->SBUF copies.

Recurrence is kept in pre-abs form r_t = proj_t + HH*h_{t-1} (h_t = |r_t|), so
each step is ONE fused DVE op:  r_new = (r_old abs_max 0.0) + proj_slice
via scalar_tensor_tensor.

Per-core device pipeline over 8 time-blocks of 32 steps:
  DMA xt block (one 1MB DMA) -> PE: proj^T[h,(t,b)] f32r matmuls into 2-bank
  PSUM tiles (c-pairs) -> ACT: strided copy to SBUF t-major proj buffer ->
  DVE: 32 fused recurrence steps.
Final: |r_T| cast to f32r in one op, then Y = h^T . W_HO^T + bias as a
17-matmul PSUM accumulation group.
"""

import numpy as np

import concourse.bass as bass
import concourse.mybir as mybir
from concourse import bacc
from concourse.alu_op_type import AluOpType
from concourse.tile import TileContext
from concourse.bass_utils import run_bass_kernel_spmd

# Problem shape (hardcoded per contract).
T, B, I, H, O = 256, 128, 512, 2048, 512
NCORES = 8
BL = B // NCORES            # 16 batch elements per core
TBLK = 32                   # time steps per block
NBLK = T // TBLK            # 8 blocks
NC_H = H // 128             # 16 h-chunks
NC_I = I // 128             # 4 i-chunks
CPAIR = 4                   # h-chunks per PSUM tile (4 banks)
F32 = mybir.dt.float32
F32R = mybir.dt.float32r
I32 = mybir.dt.int32


def _build(apply_hh: bool, repeat: int = 1):
    nc = bacc.Bacc("TRN2", target_bir_lowering=False, debug=False)

    xt = nc.dram_tensor("xt", [I, T * BL], F32R, kind="ExternalInput")
    wih_t = nc.dram_tensor("wih_t", [I, H], F32R, kind="ExternalInput")
    who_t = nc.dram_tensor("who_t", [H, O], F32R, kind="ExternalInput")
    bias = nc.dram_tensor("bias", [1, O], F32R, kind="ExternalInput")
    ones = nc.dram_tensor("ones", [1, BL], F32R, kind="ExternalInput")
    if apply_hh:
        hhb = nc.dram_tensor("hhb", [128, NC_H * BL], F32, kind="ExternalInput")
    y = nc.dram_tensor("y", [BL, O], F32, kind="ExternalOutput")

    xt3 = xt.rearrange("(ic p) f -> p ic f", ic=NC_I, p=128)
    who3 = who_t.rearrange("(c p) f -> p c f", c=NC_H, p=128)

    with TileContext(nc) as tc:
        with (
            tc.tile_pool(name="wpool", bufs=1) as wpool,
            tc.tile_pool(name="xpool", bufs=2) as xpool,
            tc.tile_pool(name="ppool", bufs=2) as ppool,
            tc.tile_pool(name="spool", bufs=1) as spool,
            tc.tile_pool(name="psum", bufs=2, space="PSUM") as psum,
        ):
            # --- weights, bias, constants (resident) ---
            wih_sb = []
            for ic in range(NC_I):
                w = wpool.tile([128, H], F32R, tag=f"wih{ic}")
                nc.sync.dma_start(out=w, in_=wih_t[ic * 128:(ic + 1) * 128, :])
                wih_sb.append(w)
            whot_sb = wpool.tile([128, NC_H * O], F32R, tag="whot")
            nc.sync.dma_start(
                out=whot_sb.rearrange("p (c f) -> p c f", c=NC_H), in_=who3
            )
            bias_sb = wpool.tile([1, O], F32R, tag="bias")
            nc.sync.dma_start(out=bias_sb, in_=bias[:, :])
            ones_sb = wpool.tile([1, BL], F32R, tag="ones")
            nc.sync.dma_start(out=ones_sb, in_=ones[:, :])
            if apply_hh:
                hhb_sb = wpool.tile([128, NC_H * BL], F32, tag="hhb")
                nc.sync.dma_start(out=hhb_sb, in_=hhb[:, :])

            # --- recurrence state (pre-abs): [128, (c, b)]; h = c*128 + p ---
            sA = spool.tile([128, NC_H * BL], F32, tag="sA")
            sB = spool.tile([128, NC_H * BL], F32, tag="sB")

            for _rep in range(repeat):
                nc.vector.memset(sA, 0.0)
                states = [sA, sB]
                for tb in range(NBLK):
                    # proj buffer, t-major: free index = t*256 + c*16 + b
                    proj = ppool.tile([128, TBLK * NC_H * BL], F32, tag="proj")
                    proj3 = proj.rearrange(
                        "p (t cb) -> p t cb", t=TBLK, cb=NC_H * BL
                    )
                    xtile = xpool.tile([128, NC_I * TBLK * BL], F32R, tag="xt")
                    xtile3 = xtile.rearrange("p (ic f) -> p ic f", ic=NC_I)
                    nc.sync.dma_start(
                        out=xtile3,
                        in_=xt3[:, :, tb * TBLK * BL:(tb + 1) * TBLK * BL],
                    )
                    for cp in range(NC_H // CPAIR):
                        ps = psum.tile([128, CPAIR * TBLK * BL], F32, tag="mm")
                        for cc in range(CPAIR):
                            c = cp * CPAIR + cc
                            for ic in range(NC_I):
                                nc.tensor.matmul(
                                    out=ps[:, cc * TBLK * BL:(cc + 1) * TBLK * BL],
                                    lhsT=wih_sb[ic][:, c * 128:(c + 1) * 128],
                                    rhs=xtile3[:, ic, :],
                                    start=(ic == 0),
                                    stop=(ic == NC_I - 1),
                                )
                        # PSUM [128, (cc, t, b)] -> SBUF t-major slice
                        # out free dims: [t stride NC_H*BL][ccb 2*16 stride 1]
                        nc.scalar.copy(
                            out=proj3[
                                :, :, cp * CPAIR * BL:(cp + 1) * CPAIR * BL
                            ].rearrange("p t (cc b) -> p t cc b", cc=CPAIR, b=BL),
                            in_=ps.rearrange(
                                "p (cc t b) -> p cc t b", cc=CPAIR, t=TBLK, b=BL
                            ).transpose([0, 2, 1, 3]),
                        )
                    for tl in range(TBLK):
                        src, dst = states
                        if apply_hh:
                            # general path: h' = |hh*h + p|
                            nc.vector.tensor_mul(out=dst, in0=src, in1=hhb_sb)
                            nc.vector.tensor_add(
                                out=dst, in0=dst, in1=proj3[:, tl, :]
                            )
                            nc.vector.tensor_scalar(
                                out=src.bitcast(I32), in0=dst.bitcast(I32),
                                scalar1=0x7FFFFFFF, scalar2=None,
                                op0=AluOpType.bitwise_and,
                            )
                            states = [src, dst]
                            continue
                        else:
                            # h' = |h + p|: TT add then sign-bit clear
                            nc.vector.tensor_add(
                                out=dst, in0=src, in1=proj3[:, tl, :]
                            )
                            nc.vector.tensor_scalar(
                                out=src.bitcast(I32), in0=dst.bitcast(I32),
                                scalar1=0x7FFFFFFF, scalar2=None,
                                op0=AluOpType.bitwise_and,
                            )
                            states = [src, dst]  # abs wrote back into src
                            continue

                # state already post-abs (h_T); cast-copy to f32r
                rfin = states[0]
                sAr = spool.tile([128, NC_H * BL], F32R, tag="sar")
                nc.vector.tensor_copy(out=sAr, in_=rfin)
                sA3 = sAr.rearrange("p (c b) -> p c b", c=NC_H, b=BL)
                yps = psum.tile([BL, O], F32, tag="mm")
                for c in range(NC_H):
                    nc.tensor.matmul(
                        out=yps,
                        lhsT=sA3[:, c, :],
                        rhs=whot_sb[:, c * O:(c + 1) * O],
                        start=(c == 0),
                        stop=False,
                    )
                nc.tensor.matmul(
                    out=yps, lhsT=ones_sb, rhs=bias_sb, start=False, stop=True,
                )
                y_sb = spool.tile([BL, O], F32, tag="y")
                nc.vector.tensor_copy(out=y_sb, in_=yps)
                nc.sync.dma_start(out=y[:, :], in_=y_sb)

    nc.compile()
    return nc


def kernel(X, W_IH, HH, W_HO, b_HO, _cache={}):
    X = np.asarray(X, dtype=np.float32)
    W_IH = np.asarray(W_IH, dtype=np.float32)
    HH = np.asarray(HH, dtype=np.float32)
    W_HO = np.asarray(W_HO, dtype=np.float32)
    b_HO = np.asarray(b_HO, dtype=np.float32)

    apply_hh = not np.all(HH == 1.0)

    if ("nc", apply_hh) not in _cache:
        _cache[("nc", apply_hh)] = _build(apply_hh)
    nc = _cache[("nc", apply_hh)]

    wih_t = np.ascontiguousarray(W_IH.T)                 # [I, H]
    who_t = np.ascontiguousarray(W_HO.T)                 # [H, O]
    bias = b_HO.reshape(1, O)
    common = {"wih_t": wih_t, "who_t": who_t, "bias": bias,
              "ones": np.ones((1, BL), dtype=np.float32)}
    if apply_hh:
        # hhb[p, c*BL + b] = HH[c*128 + p]
        hhb = np.repeat(
            HH.reshape(NC_H, 128).T[:, :, None], BL, axis=2
        ).reshape(128, NC_H * BL)
        common["hhb"] = np.ascontiguousarray(hhb)

    in_maps = []
    for k in range(NCORES):
        xk = X[:, k * BL:(k + 1) * BL, :]                # [T, BL, I]
        xt = np.ascontiguousarray(xk.transpose(2, 0, 1)).reshape(I, T * BL)
        in_maps.append({"xt": xt, **common})

    res = run_bass_kernel_spmd(nc, in_maps, core_ids=list(range(NCORES)))
    out = np.concatenate([res.results[k]["y"] for k in range(NCORES)], axis=0)
    return out.astype(np.float32)
